# revision 31
# baseline (speedup 1.0000x reference)
"""AttentionPooling (segment softmax + weighted segment sum) on 8 trn2 cores.

Strategy: shard whole segments across cores (sorted batch -> contiguous node
ranges), pad each core's slice to a common node count, run one SPMD Bass/Tile
program.  HBM traffic is minimized by uploading x twice in bf16 from the host,
both pre-tiled so every chunk DMA lands as 128 large contiguous descriptors:
``x`` [p, t*264] node-major (ch 0-255 = features, ch 256 = 1.0 for the
denominator) feeding the weighted-sum matmul as the moving operand, and
``xt`` [p, h, n] channel-major feeding the MLP matmul.  No on-chip cast,
transpose, or DRAM bounce.  Chunks are 4096 nodes with a 2048 tail (variable
sizes, so padding stays under one tile row).  Per chunk: two loads on the
Sync HWDGE ring (the ACT ring would serialize behind tanh), MLP matmuls in
weight-load waves of four 512-slices over 4 PSUM banks, tanh on ACT,
batched score matmuls (h-tile x W2, node-partitioned), then per half-chunk
exp->onehot(segment)*e on ACT+DVE over a host-computed 32-segment active
window (is_equal hoisted ahead of the exp-gated mults), and finally the
batched weighted-sum matmuls (we-tile stationary, [x|1] moving)
accumulating [64 segs, 257] in PSUM one iteration later, giving the DVE
chain a full MLP+score of cover.
Softmax max-subtraction is skipped: |s| <= ||W2||_1 + |b2| ~ 28, exp stays
in fp32 range.
"""

from contextlib import ExitStack

import numpy as np
import ml_dtypes

import concourse.bass as bass
import concourse.bacc as bacc
import concourse.tile as tile
from concourse import mybir
from concourse.bass_utils import run_bass_kernel_spmd

N_CORES = 8
NUM_GRAPHS = 512
SEGS_PER_CORE = NUM_GRAPHS // N_CORES  # 64
D = 256          # in channels
DA = 264         # in channels + ones col + pad (16B-aligned rows)
DW = 257         # wsum moving width (features + ones)
H = 128          # hidden
P = 128          # partitions
TILE_N = 128     # nodes per weight tile
CT_MAIN = 32     # tiles per main chunk (4096 nodes)
CT_TAIL = 16     # tiles per tail chunk (2048 nodes)
WARMUP_MM = 40   # dummy matmuls to warm the PE HAM clock gate

_BF16 = mybir.dt.bfloat16
_F32 = mybir.dt.float32


def _chunk_tiles(nt):
    """Split nt tiles into chunks of CT_MAIN with CT_TAIL tails."""
    cts = []
    rem = nt
    while rem >= CT_MAIN:
        cts.append(CT_MAIN)
        rem -= CT_MAIN
    while rem > 0:
        cts.append(min(CT_TAIL, rem))
        rem -= min(CT_TAIL, rem)
    return cts


def _build_program(nt: int, b2_val: float, windows=None):
    """windows: per-chunk (W0, w) active-segment window for the one-hot;
    chunk 0 must be (0, 64) so the first accumulation matmul initializes
    every PSUM element."""
    nc = bacc.Bacc()
    nmax = nt * TILE_N
    cts = _chunk_tiles(nt)
    n_chunks = len(cts)
    t0s = np.cumsum([0] + cts).tolist()
    if windows is None:
        windows = [(0, SEGS_PER_CORE)] * n_chunks
    windows = list(windows)
    windows[0] = (0, SEGS_PER_CORE)

    x_d = nc.declare_dram_parameter("x", [P, nt * DA], _BF16, isOutput=False)
    xt_d = nc.declare_dram_parameter("xt", [P, 2, nmax], _BF16, isOutput=False)
    bt_d = nc.declare_dram_parameter("batch_t", [P, nt + SEGS_PER_CORE], _BF16, isOutput=False)
    w1_d = nc.declare_dram_parameter("w1", [D, H], _BF16, isOutput=False)
    w2_d = nc.declare_dram_parameter("w2", [H, 1], _BF16, isOutput=False)
    b1_d = nc.declare_dram_parameter("b1", [H, 1], _F32, isOutput=False)
    out_d = nc.declare_dram_parameter("out_g", [SEGS_PER_CORE, D], _F32, isOutput=True)

    with tile.TileContext(nc) as tc, ExitStack() as ctx:
        const_pool = ctx.enter_context(tc.tile_pool(name="consts", bufs=1))
        x_pool = ctx.enter_context(tc.tile_pool(name="x", bufs=4))
        xt_pool = ctx.enter_context(tc.tile_pool(name="xt", bufs=4))
        h_pool = ctx.enter_context(tc.tile_pool(name="h", bufs=2))
        we_pool = ctx.enter_context(tc.tile_pool(name="we", bufs=2))
        ecol_pool = ctx.enter_context(tc.tile_pool(name="ecol", bufs=3))
        fin_pool = ctx.enter_context(tc.tile_pool(name="fin", bufs=1))
        psum_h = ctx.enter_context(
            tc.tile_pool(name="psum_h", bufs=1, space=bass.MemorySpace.PSUM))
        psum_s = ctx.enter_context(
            tc.tile_pool(name="psum_s", bufs=1, space=bass.MemorySpace.PSUM))
        psum_acc = ctx.enter_context(
            tc.tile_pool(name="psum_acc", bufs=1, space=bass.MemorySpace.PSUM))
        psum_w = ctx.enter_context(
            tc.tile_pool(name="psum_w", bufs=1, space=bass.MemorySpace.PSUM))

        # ---- constants / weights ----
        w1_sb = const_pool.tile([P, 2, H], _BF16, tag="w1")   # [:, 0, :]=ch 0-127
        nc.gpsimd.dma_start(w1_sb[:, 0, :], w1_d[0:128, :])
        nc.gpsimd.dma_start(w1_sb[:, 1, :], w1_d[128:256, :])
        w2_sb = const_pool.tile([P, 1], _BF16, tag="w2")
        nc.gpsimd.dma_start(w2_sb[:], w2_d[:])
        b1_sb = const_pool.tile([P, 1], _F32, tag="b1")
        nc.gpsimd.dma_start(b1_sb[:], b1_d[:])
        bt_sb = const_pool.tile([P, nt + SEGS_PER_CORE], _BF16, tag="bt")
        nc.gpsimd.dma_start(bt_sb[:], bt_d[:])
        iota_sb = bt_sb[:, nt:nt + SEGS_PER_CORE]

        acc_ps = psum_acc.tile([SEGS_PER_CORE, DW], _F32, tag="acc")

        # warm the HAM clock gate while the first chunk loads
        warm_ps = psum_w.tile([P, H], _F32, tag="warm")
        for _ in range(WARMUP_MM):
            nc.tensor.matmul(warm_ps[:], w1_sb[:, 0, :], w1_sb[:, 1, :],
                             start=True, stop=True)

        saved = {}

        def emit_load_mlp(c):
            t0, ct = t0s[c], cts[c]
            cn = ct * TILE_N
            # xt on the Sync HWDGE ring (latency-critical: the MLP consumes
            # it first); x on the idle GPSIMD SWDGE ring so the two streams
            # cover each other's inter-transfer seams.  The Scalar ring is
            # avoided: it shares the ACT queue and serializes behind tanh.
            xt_sb = xt_pool.tile([P, 2, CT_MAIN * TILE_N], _BF16, tag="xt")
            nc.sync.dma_start(
                xt_sb[:, :, 0:cn], xt_d[:, :, t0 * TILE_N:t0 * TILE_N + cn])
            x_sb = x_pool.tile([P, CT_MAIN, DA], _BF16, tag="x")
            nc.gpsimd.dma_start(
                x_sb[:, 0:ct, :].rearrange("p t ch -> p (t ch)"),
                x_d[:, t0 * DA:(t0 + ct) * DA])

            # h = tanh(x @ W1 + b1), hidden-partitioned, bf16.
            # Weight-load waves of 4 slices over 4 PSUM banks.
            h_bf = h_pool.tile([P, CT_MAIN * TILE_N], _BF16, tag="h")
            for w in range(cn // 2048):
                phs = [psum_h.tile([P, 512], _F32, tag=f"ph{i}", name=f"ph{i}")
                       for i in range(4)]
                for i, ph in enumerate(phs):
                    sl = slice((w * 4 + i) * 512, (w * 4 + i + 1) * 512)
                    nc.tensor.matmul(ph[:], w1_sb[:, 0, :], xt_sb[:, 0, sl],
                                     start=True, stop=False)
                for i, ph in enumerate(phs):
                    sl = slice((w * 4 + i) * 512, (w * 4 + i + 1) * 512)
                    nc.tensor.matmul(ph[:], w1_sb[:, 1, :], xt_sb[:, 1, sl],
                                     start=False, stop=True)
                for i, ph in enumerate(phs):
                    sl = slice((w * 4 + i) * 512, (w * 4 + i + 1) * 512)
                    nc.scalar.activation(h_bf[:, sl], ph[:],
                                         mybir.ActivationFunctionType.Tanh,
                                         bias=b1_sb[:])
            saved[c] = (x_sb, h_bf)

        def emit_score(c):
            t0, ct = t0s[c], cts[c]
            x_sb, h_bf = saved.pop(c)
            w0, wN = windows[c]
            we = we_pool.tile([P, CT_MAIN, SEGS_PER_CORE], _BF16, tag="we")
            hct = ct // 2
            ecols = []
            for half in range(2):
                # separate PSUM tile per half: no write-after-read with the
                # other half's exp (tile-granularity dependency tracking)
                ps_s = psum_s.tile([P, CT_MAIN // 2], _F32,
                                   tag=f"ps_s{half}", name="ps_s")
                for t in range(hct):
                    tg = half * hct + t
                    nc.tensor.matmul(ps_s[:, t:t + 1],
                                     h_bf[:, tg * TILE_N:(tg + 1) * TILE_N],
                                     w2_sb, start=True, stop=True)
                # e = exp(s + b2)  (node-partitioned, bf16)
                e_col = ecol_pool.tile([P, CT_MAIN // 2], _BF16, tag="ecol")
                nc.scalar.activation(e_col[:, 0:hct], ps_s[:, 0:hct],
                                     mybir.ActivationFunctionType.Exp,
                                     bias=float(b2_val))
                ecols.append(e_col)
            # we[p, t, g] = (batch_t == g+w0) * e, g in [0, wN): all-bf16 on
            # DVE.  Both is_equal ops are emitted ahead of the exp-gated
            # mults so they drain during earlier slack.
            cmps = []
            for half in range(2):
                cmp = we_pool.tile([P, CT_MAIN // 2, SEGS_PER_CORE], _BF16,
                                   tag=f"cmp{half}", name="cmp")
                bt_c = bt_sb[:, t0 + half * hct:t0 + (half + 1) * hct]
                nc.vector.tensor_tensor(
                    cmp[:, 0:hct, 0:wN],
                    bt_c.unsqueeze(2).broadcast_to([P, hct, wN]),
                    iota_sb[:, w0:w0 + wN].unsqueeze(1).broadcast_to(
                        [P, hct, wN]),
                    mybir.AluOpType.is_equal)
                cmps.append(cmp)
            for half in range(2):
                tsl = slice(half * hct, (half + 1) * hct)
                nc.vector.tensor_tensor(
                    we[:, tsl, 0:wN], cmps[half][:, 0:hct, 0:wN],
                    ecols[half][:, 0:hct].unsqueeze(2).broadcast_to(
                        [P, hct, wN]),
                    mybir.AluOpType.mult)
            saved[("w", c)] = (we, x_sb)

        def emit_wsum(c, is_first, is_last):
            ct = cts[c]
            w0, wN = windows[c]
            we_p, x_p = saved.pop(("w", c))
            for t in range(ct):
                nc.tensor.matmul(acc_ps[w0:w0 + wN, :], we_p[:, t, 0:wN],
                                 x_p[:, t, 0:DW],
                                 start=(is_first and t == 0),
                                 stop=(is_last and t == ct - 1),
                                 skip_group_check=True)

        emit_load_mlp(0)
        for c in range(n_chunks):
            if c + 1 < n_chunks:
                emit_load_mlp(c + 1)
            emit_score(c)
            if c >= 1:
                emit_wsum(c - 1, is_first=(c == 1), is_last=False)
        emit_wsum(n_chunks - 1, is_first=(n_chunks == 1), is_last=True)

        # ---- epilogue: out = acc[:, 0:256] / acc[:, 256] ----
        den_sb = fin_pool.tile([SEGS_PER_CORE, 1], _F32, tag="den_sb")
        nc.vector.tensor_scalar_add(den_sb[:], acc_ps[:, D:D + 1], 1e-30)
        rec_sb = fin_pool.tile([SEGS_PER_CORE, 1], _F32, tag="rec_sb")
        nc.vector.reciprocal(rec_sb[:], den_sb[:])
        out_sb = fin_pool.tile([SEGS_PER_CORE, D], _F32, tag="out_sb")
        nc.vector.tensor_scalar_mul(out_sb[:], acc_ps[:, 0:D], rec_sb[:])
        nc.sync.dma_start(out_d[:], out_sb[:])

    return nc


def _prepare_inputs(x, W1, b1, W2, b2, batch):
    batch = np.asarray(batch).astype(np.int64)
    # core k owns segments [64k, 64(k+1)); sorted batch -> contiguous ranges
    bounds = np.searchsorted(batch, np.arange(0, NUM_GRAPHS + 1, SEGS_PER_CORE))
    counts = np.diff(bounds)
    nmax = int(np.max(counts))
    nt = max(1, (nmax + TILE_N - 1) // TILE_N)
    nt = ((nt + CT_TAIL - 1) // CT_TAIL) * CT_TAIL  # multiple of 16 tiles
    nmax_pad = nt * TILE_N

    x_bf = np.asarray(x, np.float32).astype(ml_dtypes.bfloat16)
    w1_bf = np.asarray(W1, np.float32).astype(ml_dtypes.bfloat16)
    w2_bf = np.asarray(W2, np.float32).reshape(H, 1).astype(ml_dtypes.bfloat16)
    b1_col = np.asarray(b1, np.float32).reshape(H, 1)

    # per-chunk active-segment window, unioned across cores (SPMD: one
    # program).  Falls back to the full 64 when a chunk straddles a
    # 32-aligned window.
    cts = _chunk_tiles(nt)
    t0s = np.cumsum([0] + cts)
    g_lo = np.full(len(cts), 64, np.int64)
    g_hi = np.full(len(cts), -1, np.int64)
    for k in range(N_CORES):
        lo, hi = int(bounds[k]), int(bounds[k + 1])
        rel = batch[lo:hi] - k * SEGS_PER_CORE
        for c in range(len(cts)):
            n0, n1 = t0s[c] * TILE_N, t0s[c + 1] * TILE_N
            seg = rel[n0:min(n1, hi - lo)]
            if len(seg):
                g_lo[c] = min(g_lo[c], int(seg[0]))
                g_hi[c] = max(g_hi[c], int(seg[-1]))
    windows = []
    for c in range(len(cts)):
        if g_hi[c] < 0:
            windows.append((0, 32))
        elif g_lo[c] // 32 == g_hi[c] // 32:
            windows.append((32 * int(g_lo[c] // 32), 32))
        else:
            windows.append((0, SEGS_PER_CORE))

    in_maps = []
    for k in range(N_CORES):
        lo, hi = int(bounds[k]), int(bounds[k + 1])
        cnt = hi - lo
        x_pad = np.zeros((nmax_pad, DA), ml_dtypes.bfloat16)
        x_pad[:cnt, 0:D] = x_bf[lo:hi]
        x_pad[:, D] = ml_dtypes.bfloat16(1.0)
        # node-tiled: x_t[p, t, ch] = x_pad[t*128 + p, ch]
        x_tiled = np.ascontiguousarray(
            x_pad.reshape(nt, P, DA).transpose(1, 0, 2)).reshape(P, nt * DA)
        # channel-major: xt[p, h, n] = x[n, h*128 + p]
        xt_pad = np.zeros((2, P, nmax_pad), ml_dtypes.bfloat16)
        xt_pad[:, :, :cnt] = x_bf[lo:hi].T.reshape(2, P, cnt)
        xt_tiled = np.ascontiguousarray(xt_pad.transpose(1, 0, 2))
        bt = np.full((nmax_pad,), -1, np.float32)
        bt[:cnt] = (batch[lo:hi] - k * SEGS_PER_CORE).astype(np.float32)
        bt_t = bt.reshape(nt, P).T  # (128, nt)
        iota_cols = np.tile(np.arange(SEGS_PER_CORE, dtype=np.float32), (P, 1))
        bt_t = np.concatenate([bt_t, iota_cols], axis=1).astype(ml_dtypes.bfloat16)
        in_maps.append({
            "x": x_tiled,
            "xt": xt_tiled,
            "batch_t": np.ascontiguousarray(bt_t),
            "w1": w1_bf,
            "w2": w2_bf,
            "b1": b1_col,
        })
    return in_maps, nt, windows


def run(x, W1, b1, W2, b2, batch, trace=False, trace_kwargs=None):
    in_maps, nt, windows = _prepare_inputs(x, W1, b1, W2, b2, batch)
    nc = _build_program(nt, float(np.asarray(b2).reshape(-1)[0]), windows)
    nc.finalize()
    res = run_bass_kernel_spmd(nc, in_maps, list(range(N_CORES)),
                               trace=trace, **(trace_kwargs or {}))
    out = np.concatenate([np.asarray(res.results[k]["out_g"], np.float32)
                          for k in range(N_CORES)], axis=0)
    return out, res


def kernel(x, W1, b1, W2, b2, batch):
    out, _ = run(x, W1, b1, W2, b2, batch)
    return out


# revision 32
# speedup vs baseline: 1.0034x; 1.0034x over previous
"""AttentionPooling (segment softmax + weighted segment sum) on 8 trn2 cores.

Strategy: shard whole segments across cores (sorted batch -> contiguous node
ranges), pad each core's slice to a common node count, run one SPMD Bass/Tile
program.  HBM traffic is minimized by uploading x twice in bf16 from the host,
both pre-tiled so every chunk DMA lands as 128 large contiguous descriptors:
``x`` [p, t*264] node-major (ch 0-255 = features, ch 256 = 1.0 for the
denominator) feeding the weighted-sum matmul as the moving operand, and
``xt`` [p, h, n] channel-major feeding the MLP matmul.  No on-chip cast,
transpose, or DRAM bounce.  Chunks are 4096 nodes with a 2048 tail (variable
sizes, so padding stays under one tile row).  Per chunk: two HWDGE loads
(SP + ACT rings), MLP matmuls in weight-load waves of four 512-slices over
4 PSUM banks, tanh on ACT, batched score matmuls (h-tile x W2,
node-partitioned), then per half-chunk exp->onehot(segment)*e on ACT+DVE
(split to shorten the critical chain), and finally the batched weighted-sum
matmuls (we-tile stationary, [x|1] moving) accumulating [64 segs, 257] in
PSUM one iteration later, giving the DVE chain a full MLP+score of cover.
Softmax max-subtraction is skipped: |s| <= ||W2||_1 + |b2| ~ 28, exp stays
in fp32 range.
"""

from contextlib import ExitStack

import numpy as np
import ml_dtypes

import concourse.bass as bass
import concourse.bacc as bacc
import concourse.tile as tile
from concourse import mybir
from concourse.bass_utils import run_bass_kernel_spmd

N_CORES = 8
NUM_GRAPHS = 512
SEGS_PER_CORE = NUM_GRAPHS // N_CORES  # 64
D = 256          # in channels
DA = 264         # in channels + ones col + pad (16B-aligned rows)
DW = 257         # wsum moving width (features + ones)
H = 128          # hidden
P = 128          # partitions
TILE_N = 128     # nodes per weight tile
CT_MAIN = 32     # tiles per main chunk (4096 nodes)
CT_TAIL = 16     # tiles per tail chunk (2048 nodes)
WARMUP_MM = 40   # dummy matmuls to warm the PE HAM clock gate

_BF16 = mybir.dt.bfloat16
_F32 = mybir.dt.float32


def _chunk_tiles(nt):
    """Split nt tiles into chunks of CT_MAIN with CT_TAIL tails."""
    cts = []
    rem = nt
    while rem >= CT_MAIN:
        cts.append(CT_MAIN)
        rem -= CT_MAIN
    while rem > 0:
        cts.append(min(CT_TAIL, rem))
        rem -= min(CT_TAIL, rem)
    return cts


def _build_program(nt: int, b2_val: float, windows=None):
    """windows: per-chunk (W0, w) active-segment window for the one-hot;
    chunk 0 must be (0, 64) so the first accumulation matmul initializes
    every PSUM element."""
    nc = bacc.Bacc()
    nmax = nt * TILE_N
    cts = _chunk_tiles(nt)
    n_chunks = len(cts)
    t0s = np.cumsum([0] + cts).tolist()
    if windows is None:
        windows = [(0, SEGS_PER_CORE)] * n_chunks
    windows = list(windows)
    windows[0] = (0, SEGS_PER_CORE)

    x_d = nc.declare_dram_parameter("x", [P, nt * DA], _BF16, isOutput=False)
    xt_d = nc.declare_dram_parameter("xt", [P, 2, nmax], _BF16, isOutput=False)
    bt_d = nc.declare_dram_parameter("batch_t", [P, nt + SEGS_PER_CORE], _BF16, isOutput=False)
    w1_d = nc.declare_dram_parameter("w1", [D, H], _BF16, isOutput=False)
    w2_d = nc.declare_dram_parameter("w2", [H, 1], _BF16, isOutput=False)
    b1_d = nc.declare_dram_parameter("b1", [H, 1], _F32, isOutput=False)
    out_d = nc.declare_dram_parameter("out_g", [SEGS_PER_CORE, D], _F32, isOutput=True)

    with tile.TileContext(nc) as tc, ExitStack() as ctx:
        const_pool = ctx.enter_context(tc.tile_pool(name="consts", bufs=1))
        x_pool = ctx.enter_context(tc.tile_pool(name="x", bufs=4))
        xt_pool = ctx.enter_context(tc.tile_pool(name="xt", bufs=4))
        h_pool = ctx.enter_context(tc.tile_pool(name="h", bufs=2))
        we_pool = ctx.enter_context(tc.tile_pool(name="we", bufs=2))
        ecol_pool = ctx.enter_context(tc.tile_pool(name="ecol", bufs=3))
        fin_pool = ctx.enter_context(tc.tile_pool(name="fin", bufs=1))
        psum_h = ctx.enter_context(
            tc.tile_pool(name="psum_h", bufs=1, space=bass.MemorySpace.PSUM))
        psum_s = ctx.enter_context(
            tc.tile_pool(name="psum_s", bufs=1, space=bass.MemorySpace.PSUM))
        psum_acc = ctx.enter_context(
            tc.tile_pool(name="psum_acc", bufs=1, space=bass.MemorySpace.PSUM))
        psum_w = ctx.enter_context(
            tc.tile_pool(name="psum_w", bufs=1, space=bass.MemorySpace.PSUM))

        # ---- constants / weights ----
        w1_sb = const_pool.tile([P, 2, H], _BF16, tag="w1")   # [:, 0, :]=ch 0-127
        # preloads ride the Scalar ring: the ACT queue is empty this early,
        # and it frees the Sync ring head for chunk 0's data.
        nc.scalar.dma_start(w1_sb[:, 0, :], w1_d[0:128, :])
        nc.scalar.dma_start(w1_sb[:, 1, :], w1_d[128:256, :])
        w2_sb = const_pool.tile([P, 1], _BF16, tag="w2")
        nc.scalar.dma_start(w2_sb[:], w2_d[:])
        b1_sb = const_pool.tile([P, 1], _F32, tag="b1")
        nc.scalar.dma_start(b1_sb[:], b1_d[:])
        bt_sb = const_pool.tile([P, nt + SEGS_PER_CORE], _BF16, tag="bt")
        nc.scalar.dma_start(bt_sb[:], bt_d[:])
        iota_sb = bt_sb[:, nt:nt + SEGS_PER_CORE]

        acc_ps = psum_acc.tile([SEGS_PER_CORE, DW], _F32, tag="acc")

        # warm the HAM clock gate while the first chunk loads
        warm_ps = psum_w.tile([P, H], _F32, tag="warm")
        for _ in range(WARMUP_MM):
            nc.tensor.matmul(warm_ps[:], w1_sb[:, 0, :], w1_sb[:, 1, :],
                             start=True, stop=True)

        saved = {}

        def emit_load_mlp(c):
            t0, ct = t0s[c], cts[c]
            cn = ct * TILE_N
            # both loads on the Sync HWDGE ring: the Scalar ring shares the
            # ACT engine queue, where a dma_start would serialize behind the
            # previous chunk's tanh.  xt first — the MLP consumes it soonest.
            xt_sb = xt_pool.tile([P, 2, CT_MAIN * TILE_N], _BF16, tag="xt")
            nc.sync.dma_start(
                xt_sb[:, :, 0:cn], xt_d[:, :, t0 * TILE_N:t0 * TILE_N + cn])
            x_sb = x_pool.tile([P, CT_MAIN, DA], _BF16, tag="x")
            nc.sync.dma_start(
                x_sb[:, 0:ct, :].rearrange("p t ch -> p (t ch)"),
                x_d[:, t0 * DA:(t0 + ct) * DA])

            # h = tanh(x @ W1 + b1), hidden-partitioned, bf16.
            # Weight-load waves of 4 slices over 4 PSUM banks.
            h_bf = h_pool.tile([P, CT_MAIN * TILE_N], _BF16, tag="h")
            for w in range(cn // 2048):
                phs = [psum_h.tile([P, 512], _F32, tag=f"ph{i}", name=f"ph{i}")
                       for i in range(4)]
                for i, ph in enumerate(phs):
                    sl = slice((w * 4 + i) * 512, (w * 4 + i + 1) * 512)
                    nc.tensor.matmul(ph[:], w1_sb[:, 0, :], xt_sb[:, 0, sl],
                                     start=True, stop=False)
                for i, ph in enumerate(phs):
                    sl = slice((w * 4 + i) * 512, (w * 4 + i + 1) * 512)
                    nc.tensor.matmul(ph[:], w1_sb[:, 1, :], xt_sb[:, 1, sl],
                                     start=False, stop=True)
                for i, ph in enumerate(phs):
                    sl = slice((w * 4 + i) * 512, (w * 4 + i + 1) * 512)
                    nc.scalar.activation(h_bf[:, sl], ph[:],
                                         mybir.ActivationFunctionType.Tanh,
                                         bias=b1_sb[:])
            saved[c] = (x_sb, h_bf)

        def emit_score(c):
            t0, ct = t0s[c], cts[c]
            x_sb, h_bf = saved.pop(c)
            w0, wN = windows[c]
            we = we_pool.tile([P, CT_MAIN, SEGS_PER_CORE], _BF16, tag="we")
            hct = ct // 2
            ecols = []
            for half in range(2):
                # separate PSUM tile per half: no write-after-read with the
                # other half's exp (tile-granularity dependency tracking)
                ps_s = psum_s.tile([P, CT_MAIN // 2], _F32,
                                   tag=f"ps_s{half}", name="ps_s")
                for t in range(hct):
                    tg = half * hct + t
                    nc.tensor.matmul(ps_s[:, t:t + 1],
                                     h_bf[:, tg * TILE_N:(tg + 1) * TILE_N],
                                     w2_sb, start=True, stop=True)
                # e = exp(s + b2)  (node-partitioned, bf16)
                e_col = ecol_pool.tile([P, CT_MAIN // 2], _BF16, tag="ecol")
                nc.scalar.activation(e_col[:, 0:hct], ps_s[:, 0:hct],
                                     mybir.ActivationFunctionType.Exp,
                                     bias=float(b2_val))
                ecols.append(e_col)
            # we[p, t, g] = (batch_t == g+w0) * e, g in [0, wN): all-bf16 on
            # DVE.  Both is_equal ops are emitted ahead of the exp-gated
            # mults so they drain during earlier slack.
            cmps = []
            for half in range(2):
                cmp = we_pool.tile([P, CT_MAIN // 2, SEGS_PER_CORE], _BF16,
                                   tag=f"cmp{half}", name="cmp")
                bt_c = bt_sb[:, t0 + half * hct:t0 + (half + 1) * hct]
                nc.vector.tensor_tensor(
                    cmp[:, 0:hct, 0:wN],
                    bt_c.unsqueeze(2).broadcast_to([P, hct, wN]),
                    iota_sb[:, w0:w0 + wN].unsqueeze(1).broadcast_to(
                        [P, hct, wN]),
                    mybir.AluOpType.is_equal)
                cmps.append(cmp)
            for half in range(2):
                tsl = slice(half * hct, (half + 1) * hct)
                nc.vector.tensor_tensor(
                    we[:, tsl, 0:wN], cmps[half][:, 0:hct, 0:wN],
                    ecols[half][:, 0:hct].unsqueeze(2).broadcast_to(
                        [P, hct, wN]),
                    mybir.AluOpType.mult)
            saved[("w", c)] = (we, x_sb)

        def emit_wsum(c, is_first, is_last):
            ct = cts[c]
            w0, wN = windows[c]
            we_p, x_p = saved.pop(("w", c))
            for t in range(ct):
                nc.tensor.matmul(acc_ps[w0:w0 + wN, :], we_p[:, t, 0:wN],
                                 x_p[:, t, 0:DW],
                                 start=(is_first and t == 0),
                                 stop=(is_last and t == ct - 1),
                                 skip_group_check=True)

        emit_load_mlp(0)
        for c in range(n_chunks):
            if c + 1 < n_chunks:
                emit_load_mlp(c + 1)
            emit_score(c)
            if c >= 1:
                emit_wsum(c - 1, is_first=(c == 1), is_last=False)
        emit_wsum(n_chunks - 1, is_first=(n_chunks == 1), is_last=True)

        # ---- epilogue: out = acc[:, 0:256] / acc[:, 256] ----
        den_sb = fin_pool.tile([SEGS_PER_CORE, 1], _F32, tag="den_sb")
        nc.vector.tensor_scalar_add(den_sb[:], acc_ps[:, D:D + 1], 1e-30)
        rec_sb = fin_pool.tile([SEGS_PER_CORE, 1], _F32, tag="rec_sb")
        nc.vector.reciprocal(rec_sb[:], den_sb[:])
        out_sb = fin_pool.tile([SEGS_PER_CORE, D], _F32, tag="out_sb")
        nc.vector.tensor_scalar_mul(out_sb[:], acc_ps[:, 0:D], rec_sb[:])
        nc.sync.dma_start(out_d[:], out_sb[:])

    return nc


def _prepare_inputs(x, W1, b1, W2, b2, batch):
    batch = np.asarray(batch).astype(np.int64)
    # core k owns segments [64k, 64(k+1)); sorted batch -> contiguous ranges
    bounds = np.searchsorted(batch, np.arange(0, NUM_GRAPHS + 1, SEGS_PER_CORE))
    counts = np.diff(bounds)
    nmax = int(np.max(counts))
    nt = max(1, (nmax + TILE_N - 1) // TILE_N)
    nt = ((nt + CT_TAIL - 1) // CT_TAIL) * CT_TAIL  # multiple of 16 tiles
    nmax_pad = nt * TILE_N

    x_bf = np.asarray(x, np.float32).astype(ml_dtypes.bfloat16)
    w1_bf = np.asarray(W1, np.float32).astype(ml_dtypes.bfloat16)
    w2_bf = np.asarray(W2, np.float32).reshape(H, 1).astype(ml_dtypes.bfloat16)
    b1_col = np.asarray(b1, np.float32).reshape(H, 1)

    # per-chunk active-segment window, unioned across cores (SPMD: one
    # program).  Falls back to the full 64 when a chunk straddles a
    # 32-aligned window.
    cts = _chunk_tiles(nt)
    t0s = np.cumsum([0] + cts)
    g_lo = np.full(len(cts), 64, np.int64)
    g_hi = np.full(len(cts), -1, np.int64)
    for k in range(N_CORES):
        lo, hi = int(bounds[k]), int(bounds[k + 1])
        rel = batch[lo:hi] - k * SEGS_PER_CORE
        for c in range(len(cts)):
            n0, n1 = t0s[c] * TILE_N, t0s[c + 1] * TILE_N
            seg = rel[n0:min(n1, hi - lo)]
            if len(seg):
                g_lo[c] = min(g_lo[c], int(seg[0]))
                g_hi[c] = max(g_hi[c], int(seg[-1]))
    windows = []
    for c in range(len(cts)):
        if g_hi[c] < 0:
            windows.append((0, 32))
        elif g_lo[c] // 32 == g_hi[c] // 32:
            windows.append((32 * int(g_lo[c] // 32), 32))
        else:
            windows.append((0, SEGS_PER_CORE))

    in_maps = []
    for k in range(N_CORES):
        lo, hi = int(bounds[k]), int(bounds[k + 1])
        cnt = hi - lo
        x_pad = np.zeros((nmax_pad, DA), ml_dtypes.bfloat16)
        x_pad[:cnt, 0:D] = x_bf[lo:hi]
        x_pad[:, D] = ml_dtypes.bfloat16(1.0)
        # node-tiled: x_t[p, t, ch] = x_pad[t*128 + p, ch]
        x_tiled = np.ascontiguousarray(
            x_pad.reshape(nt, P, DA).transpose(1, 0, 2)).reshape(P, nt * DA)
        # channel-major: xt[p, h, n] = x[n, h*128 + p]
        xt_pad = np.zeros((2, P, nmax_pad), ml_dtypes.bfloat16)
        xt_pad[:, :, :cnt] = x_bf[lo:hi].T.reshape(2, P, cnt)
        xt_tiled = np.ascontiguousarray(xt_pad.transpose(1, 0, 2))
        bt = np.full((nmax_pad,), -1, np.float32)
        bt[:cnt] = (batch[lo:hi] - k * SEGS_PER_CORE).astype(np.float32)
        bt_t = bt.reshape(nt, P).T  # (128, nt)
        iota_cols = np.tile(np.arange(SEGS_PER_CORE, dtype=np.float32), (P, 1))
        bt_t = np.concatenate([bt_t, iota_cols], axis=1).astype(ml_dtypes.bfloat16)
        in_maps.append({
            "x": x_tiled,
            "xt": xt_tiled,
            "batch_t": np.ascontiguousarray(bt_t),
            "w1": w1_bf,
            "w2": w2_bf,
            "b1": b1_col,
        })
    return in_maps, nt, windows


def run(x, W1, b1, W2, b2, batch, trace=False, trace_kwargs=None):
    in_maps, nt, windows = _prepare_inputs(x, W1, b1, W2, b2, batch)
    nc = _build_program(nt, float(np.asarray(b2).reshape(-1)[0]), windows)
    nc.finalize()
    res = run_bass_kernel_spmd(nc, in_maps, list(range(N_CORES)),
                               trace=trace, **(trace_kwargs or {}))
    out = np.concatenate([np.asarray(res.results[k]["out_g"], np.float32)
                          for k in range(N_CORES)], axis=0)
    return out, res


def kernel(x, W1, b1, W2, b2, batch):
    out, _ = run(x, W1, b1, W2, b2, batch)
    return out


# revision 33
# speedup vs baseline: 1.2080x; 1.2039x over previous
"""AttentionPooling (segment softmax + weighted segment sum) on 8 trn2 cores.

Strategy: shard whole segments across cores (sorted batch -> contiguous node
ranges), pad each core's slice to a common node count, run one SPMD Bass/Tile
program.  HBM traffic is minimized by uploading x twice in bf16 from the host,
both pre-tiled so every chunk DMA lands as 128 large contiguous descriptors:
``x`` [p, t*264] node-major (ch 0-255 = features, ch 256 = 1.0 for the
denominator) feeding the weighted-sum matmul as the moving operand, and
``xt`` [p, h, n] channel-major feeding the MLP matmul.  No on-chip cast,
transpose, or DRAM bounce.  Chunks are 4096 nodes with a 2048 tail (variable
sizes, so padding stays under one tile row).  Per chunk: two HWDGE loads
(SP + ACT rings), MLP matmuls in weight-load waves of four 512-slices over
4 PSUM banks, tanh on ACT, batched score matmuls (h-tile x W2,
node-partitioned), then per half-chunk exp->onehot(segment)*e on ACT+DVE
(split to shorten the critical chain), and finally the batched weighted-sum
matmuls (we-tile stationary, [x|1] moving) accumulating [64 segs, 257] in
PSUM one iteration later, giving the DVE chain a full MLP+score of cover.
Softmax max-subtraction is skipped: |s| <= ||W2||_1 + |b2| ~ 28, exp stays
in fp32 range.
"""

from contextlib import ExitStack

import numpy as np
import ml_dtypes

import concourse.bass as bass
import concourse.bacc as bacc
import concourse.tile as tile
from concourse import mybir
from concourse.bass_utils import run_bass_kernel_spmd

N_CORES = 8
NUM_GRAPHS = 512
SEGS_PER_CORE = NUM_GRAPHS // N_CORES  # 64
D = 256          # in channels
DA = 264         # in channels + ones col + pad (16B-aligned rows)
DW = 257         # wsum moving width (features + ones)
H = 128          # hidden
P = 128          # partitions
TILE_N = 128     # nodes per weight tile
CT_MAIN = 32     # tiles per main chunk (4096 nodes)
CT_TAIL = 16     # tiles per tail chunk (2048 nodes)
WARMUP_MM = 40   # dummy matmuls to warm the PE HAM clock gate

_BF16 = mybir.dt.bfloat16
_F32 = mybir.dt.float32


def _chunk_tiles(nt):
    """Split nt tiles into chunks of CT_MAIN with CT_TAIL tails."""
    cts = []
    rem = nt
    while rem >= CT_MAIN:
        cts.append(CT_MAIN)
        rem -= CT_MAIN
    while rem > 0:
        cts.append(min(CT_TAIL, rem))
        rem -= min(CT_TAIL, rem)
    return cts


def _build_program(nt: int, b2_val: float, windows=None):
    """windows: per-chunk (W0, w) active-segment window for the one-hot;
    chunk 0 must be (0, 64) so the first accumulation matmul initializes
    every PSUM element."""
    nc = bacc.Bacc()
    nmax = nt * TILE_N
    cts = _chunk_tiles(nt)
    n_chunks = len(cts)
    t0s = np.cumsum([0] + cts).tolist()
    if windows is None:
        windows = [(0, SEGS_PER_CORE)] * n_chunks
    windows = list(windows)
    windows[0] = (0, SEGS_PER_CORE)

    x_d = nc.declare_dram_parameter("x", [P, nt * DA], _BF16, isOutput=False)
    xt_d = nc.declare_dram_parameter("xt", [P, 2, nmax], _BF16, isOutput=False)
    bt_d = nc.declare_dram_parameter("batch_t", [P, nt + SEGS_PER_CORE], _BF16, isOutput=False)
    w1_d = nc.declare_dram_parameter("w1", [D, H], _BF16, isOutput=False)
    w2_d = nc.declare_dram_parameter("w2", [H, 1], _BF16, isOutput=False)
    b1_d = nc.declare_dram_parameter("b1", [H, 1], _F32, isOutput=False)
    out_d = nc.declare_dram_parameter("out_g", [SEGS_PER_CORE, D], _F32, isOutput=True)

    with tile.TileContext(nc) as tc, ExitStack() as ctx:
        const_pool = ctx.enter_context(tc.tile_pool(name="consts", bufs=1))
        x_pool = ctx.enter_context(tc.tile_pool(name="x", bufs=4))
        xt_pool = ctx.enter_context(tc.tile_pool(name="xt", bufs=4))
        h_pool = ctx.enter_context(tc.tile_pool(name="h", bufs=2))
        we_pool = ctx.enter_context(tc.tile_pool(name="we", bufs=2))
        ecol_pool = ctx.enter_context(tc.tile_pool(name="ecol", bufs=3))
        fin_pool = ctx.enter_context(tc.tile_pool(name="fin", bufs=1))
        psum_h = ctx.enter_context(
            tc.tile_pool(name="psum_h", bufs=1, space=bass.MemorySpace.PSUM))
        psum_s = ctx.enter_context(
            tc.tile_pool(name="psum_s", bufs=1, space=bass.MemorySpace.PSUM))
        psum_acc = ctx.enter_context(
            tc.tile_pool(name="psum_acc", bufs=1, space=bass.MemorySpace.PSUM))
        psum_w = ctx.enter_context(
            tc.tile_pool(name="psum_w", bufs=1, space=bass.MemorySpace.PSUM))

        # ---- constants / weights ----
        w1_sb = const_pool.tile([P, 2, H], _BF16, tag="w1")   # [:, 0, :]=ch 0-127
        nc.sync.dma_start(w1_sb[:, 0, :], w1_d[0:128, :])
        nc.sync.dma_start(w1_sb[:, 1, :], w1_d[128:256, :])
        w2_sb = const_pool.tile([P, 1], _BF16, tag="w2")
        nc.sync.dma_start(w2_sb[:], w2_d[:])
        b1_sb = const_pool.tile([P, 1], _F32, tag="b1")
        nc.sync.dma_start(b1_sb[:], b1_d[:])
        bt_sb = const_pool.tile([P, nt + SEGS_PER_CORE], _BF16, tag="bt")
        nc.sync.dma_start(bt_sb[:], bt_d[:])
        iota_sb = bt_sb[:, nt:nt + SEGS_PER_CORE]

        acc_ps = psum_acc.tile([SEGS_PER_CORE, DW], _F32, tag="acc")

        # warm the HAM clock gate while the first chunk loads
        warm_ps = psum_w.tile([P, H], _F32, tag="warm")
        for _ in range(WARMUP_MM):
            nc.tensor.matmul(warm_ps[:], w1_sb[:, 0, :], w1_sb[:, 1, :],
                             start=True, stop=True)

        saved = {}

        def emit_load_mlp(c):
            t0, ct = t0s[c], cts[c]
            cn = ct * TILE_N
            # both loads on the Sync HWDGE ring: the Scalar ring shares the
            # ACT engine queue, where a dma_start would serialize behind the
            # previous chunk's tanh.  xt first — the MLP consumes it soonest.
            xt_sb = xt_pool.tile([P, 2, CT_MAIN * TILE_N], _BF16, tag="xt")
            nc.sync.dma_start(
                xt_sb[:, :, 0:cn], xt_d[:, :, t0 * TILE_N:t0 * TILE_N + cn])
            x_sb = x_pool.tile([P, CT_MAIN, DA], _BF16, tag="x")
            nc.sync.dma_start(
                x_sb[:, 0:ct, :].rearrange("p t ch -> p (t ch)"),
                x_d[:, t0 * DA:(t0 + ct) * DA])

            # h = tanh(x @ W1 + b1), hidden-partitioned, bf16.
            # Weight-load waves of 4 slices over 4 PSUM banks.
            h_bf = h_pool.tile([P, CT_MAIN * TILE_N], _BF16, tag="h")
            for w in range(cn // 2048):
                phs = [psum_h.tile([P, 512], _F32, tag=f"ph{i}", name=f"ph{i}")
                       for i in range(4)]
                for i, ph in enumerate(phs):
                    sl = slice((w * 4 + i) * 512, (w * 4 + i + 1) * 512)
                    nc.tensor.matmul(ph[:], w1_sb[:, 0, :], xt_sb[:, 0, sl],
                                     start=True, stop=False)
                for i, ph in enumerate(phs):
                    sl = slice((w * 4 + i) * 512, (w * 4 + i + 1) * 512)
                    nc.tensor.matmul(ph[:], w1_sb[:, 1, :], xt_sb[:, 1, sl],
                                     start=False, stop=True)
                for i, ph in enumerate(phs):
                    sl = slice((w * 4 + i) * 512, (w * 4 + i + 1) * 512)
                    nc.scalar.activation(h_bf[:, sl], ph[:],
                                         mybir.ActivationFunctionType.Tanh,
                                         bias=b1_sb[:])
            saved[c] = (x_sb, h_bf)

        def emit_score(c):
            t0, ct = t0s[c], cts[c]
            x_sb, h_bf = saved.pop(c)
            w0, wN = windows[c]
            we = we_pool.tile([P, CT_MAIN, SEGS_PER_CORE], _BF16, tag="we")
            hct = ct // 2
            ecols = []
            for half in range(2):
                # separate PSUM tile per half: no write-after-read with the
                # other half's exp (tile-granularity dependency tracking)
                ps_s = psum_s.tile([P, CT_MAIN // 2], _F32,
                                   tag=f"ps_s{half}", name="ps_s")
                for t in range(hct):
                    tg = half * hct + t
                    nc.tensor.matmul(ps_s[:, t:t + 1],
                                     h_bf[:, tg * TILE_N:(tg + 1) * TILE_N],
                                     w2_sb, start=True, stop=True)
                # e = exp(s + b2)  (node-partitioned, bf16)
                e_col = ecol_pool.tile([P, CT_MAIN // 2], _BF16, tag="ecol")
                nc.scalar.activation(e_col[:, 0:hct], ps_s[:, 0:hct],
                                     mybir.ActivationFunctionType.Exp,
                                     bias=float(b2_val))
                ecols.append(e_col)
            # we[p, t, g] = (batch_t == g+w0) * e, g in [0, wN): all-bf16 on
            # DVE.  Both is_equal ops are emitted ahead of the exp-gated
            # mults so they drain during earlier slack.
            cmps = []
            for half in range(2):
                cmp = we_pool.tile([P, CT_MAIN // 2, SEGS_PER_CORE], _BF16,
                                   tag=f"cmp{half}", name="cmp")
                bt_c = bt_sb[:, t0 + half * hct:t0 + (half + 1) * hct]
                nc.vector.tensor_tensor(
                    cmp[:, 0:hct, 0:wN],
                    bt_c.unsqueeze(2).broadcast_to([P, hct, wN]),
                    iota_sb[:, w0:w0 + wN].unsqueeze(1).broadcast_to(
                        [P, hct, wN]),
                    mybir.AluOpType.is_equal)
                cmps.append(cmp)
            for half in range(2):
                tsl = slice(half * hct, (half + 1) * hct)
                nc.vector.tensor_tensor(
                    we[:, tsl, 0:wN], cmps[half][:, 0:hct, 0:wN],
                    ecols[half][:, 0:hct].unsqueeze(2).broadcast_to(
                        [P, hct, wN]),
                    mybir.AluOpType.mult)
            saved[("w", c)] = (we, x_sb)

        def emit_wsum(c, is_first, is_last):
            ct = cts[c]
            w0, wN = windows[c]
            we_p, x_p = saved.pop(("w", c))
            for t in range(ct):
                nc.tensor.matmul(acc_ps[w0:w0 + wN, :], we_p[:, t, 0:wN],
                                 x_p[:, t, 0:DW],
                                 start=(is_first and t == 0),
                                 stop=(is_last and t == ct - 1),
                                 skip_group_check=True)

        emit_load_mlp(0)
        for c in range(n_chunks):
            if c + 1 < n_chunks:
                emit_load_mlp(c + 1)
            emit_score(c)
            if c >= 1:
                emit_wsum(c - 1, is_first=(c == 1), is_last=False)
        emit_wsum(n_chunks - 1, is_first=(n_chunks == 1), is_last=True)

        # ---- epilogue: out = acc[:, 0:256] / acc[:, 256] ----
        den_sb = fin_pool.tile([SEGS_PER_CORE, 1], _F32, tag="den_sb")
        nc.vector.tensor_scalar_add(den_sb[:], acc_ps[:, D:D + 1], 1e-30)
        rec_sb = fin_pool.tile([SEGS_PER_CORE, 1], _F32, tag="rec_sb")
        nc.vector.reciprocal(rec_sb[:], den_sb[:])
        out_sb = fin_pool.tile([SEGS_PER_CORE, D], _F32, tag="out_sb")
        nc.vector.tensor_scalar_mul(out_sb[:], acc_ps[:, 0:D], rec_sb[:])
        nc.sync.dma_start(out_d[:], out_sb[:])

    return nc


def _prepare_inputs(x, W1, b1, W2, b2, batch):
    batch = np.asarray(batch).astype(np.int64)
    # core k owns segments [64k, 64(k+1)); sorted batch -> contiguous ranges
    bounds = np.searchsorted(batch, np.arange(0, NUM_GRAPHS + 1, SEGS_PER_CORE))
    counts = np.diff(bounds)
    nmax = int(np.max(counts))
    nt = max(1, (nmax + TILE_N - 1) // TILE_N)
    nt = ((nt + CT_TAIL - 1) // CT_TAIL) * CT_TAIL  # multiple of 16 tiles
    nmax_pad = nt * TILE_N

    x_bf = np.asarray(x, np.float32).astype(ml_dtypes.bfloat16)
    w1_bf = np.asarray(W1, np.float32).astype(ml_dtypes.bfloat16)
    w2_bf = np.asarray(W2, np.float32).reshape(H, 1).astype(ml_dtypes.bfloat16)
    b1_col = np.asarray(b1, np.float32).reshape(H, 1)

    # per-chunk active-segment window, unioned across cores (SPMD: one
    # program).  Falls back to the full 64 when a chunk straddles a
    # 32-aligned window.
    cts = _chunk_tiles(nt)
    t0s = np.cumsum([0] + cts)
    g_lo = np.full(len(cts), 64, np.int64)
    g_hi = np.full(len(cts), -1, np.int64)
    for k in range(N_CORES):
        lo, hi = int(bounds[k]), int(bounds[k + 1])
        rel = batch[lo:hi] - k * SEGS_PER_CORE
        for c in range(len(cts)):
            n0, n1 = t0s[c] * TILE_N, t0s[c + 1] * TILE_N
            seg = rel[n0:min(n1, hi - lo)]
            if len(seg):
                g_lo[c] = min(g_lo[c], int(seg[0]))
                g_hi[c] = max(g_hi[c], int(seg[-1]))
    windows = []
    for c in range(len(cts)):
        if g_hi[c] < 0:
            windows.append((0, 32))
        elif g_lo[c] // 32 == g_hi[c] // 32:
            windows.append((32 * int(g_lo[c] // 32), 32))
        else:
            windows.append((0, SEGS_PER_CORE))

    in_maps = []
    for k in range(N_CORES):
        lo, hi = int(bounds[k]), int(bounds[k + 1])
        cnt = hi - lo
        x_pad = np.zeros((nmax_pad, DA), ml_dtypes.bfloat16)
        x_pad[:cnt, 0:D] = x_bf[lo:hi]
        x_pad[:, D] = ml_dtypes.bfloat16(1.0)
        # node-tiled: x_t[p, t, ch] = x_pad[t*128 + p, ch]
        x_tiled = np.ascontiguousarray(
            x_pad.reshape(nt, P, DA).transpose(1, 0, 2)).reshape(P, nt * DA)
        # channel-major: xt[p, h, n] = x[n, h*128 + p]
        xt_pad = np.zeros((2, P, nmax_pad), ml_dtypes.bfloat16)
        xt_pad[:, :, :cnt] = x_bf[lo:hi].T.reshape(2, P, cnt)
        xt_tiled = np.ascontiguousarray(xt_pad.transpose(1, 0, 2))
        bt = np.full((nmax_pad,), -1, np.float32)
        bt[:cnt] = (batch[lo:hi] - k * SEGS_PER_CORE).astype(np.float32)
        bt_t = bt.reshape(nt, P).T  # (128, nt)
        iota_cols = np.tile(np.arange(SEGS_PER_CORE, dtype=np.float32), (P, 1))
        bt_t = np.concatenate([bt_t, iota_cols], axis=1).astype(ml_dtypes.bfloat16)
        in_maps.append({
            "x": x_tiled,
            "xt": xt_tiled,
            "batch_t": np.ascontiguousarray(bt_t),
            "w1": w1_bf,
            "w2": w2_bf,
            "b1": b1_col,
        })
    return in_maps, nt, windows


def run(x, W1, b1, W2, b2, batch, trace=False, trace_kwargs=None):
    in_maps, nt, windows = _prepare_inputs(x, W1, b1, W2, b2, batch)
    nc = _build_program(nt, float(np.asarray(b2).reshape(-1)[0]), windows)
    nc.finalize()
    res = run_bass_kernel_spmd(nc, in_maps, list(range(N_CORES)),
                               trace=trace, **(trace_kwargs or {}))
    out = np.concatenate([np.asarray(res.results[k]["out_g"], np.float32)
                          for k in range(N_CORES)], axis=0)
    return out, res


def kernel(x, W1, b1, W2, b2, batch):
    out, _ = run(x, W1, b1, W2, b2, batch)
    return out
